# revision 32
# baseline (speedup 1.0000x reference)
"""Trainium2 Bass kernel for nn_CVLFuser (retrieval KNN fuser), v2.

out = silu(concat([1.0*C, 0.5*K, 0.25*T], axis=1)) where T is the
softmax(-cdist/temp)-weighted sum of the top_k nearest tie_kb rows to
q = C @ Q_weight.T.

Sharding: data-parallel over the batch dim across 8 NeuronCores; tie_kb
replicated. Per core (512 rows x 65536 KB):

Phase 1 (per 1024-col kb chunk x 128-row tile):
  - u = 2q.kb + (d - kb_sq) via fp8 DoubleRow matmuls; the (d - kb_sq)
    bias rides as one extra DoubleRow pair with fp8 planes (32*hi + lo).
  - One ACT pass evacuates the [128,1024] PSUM as fp16
    relu(s*u + B) -- integer-valued in [1024, 2046] (10-bit quantizer).
  - DVE steals the 2 value LSBs as a quad-id (tensor_scalar and/or at
    4x), reduces 4 quads with a tensor_tensor max tree (2x), then runs
    Max/MaxIndex on just [128,256]: top-8 per chunk with the full index
    recoverable as c*1024 + quad*256 + pos.
Phase 2 (per row tile): pack candidates as
  (v-1024)*16384 + c*256 + pos  (exact f32 ints < 2^24), merge top-32
  via Max/match_replace rounds, unpack bits to global indices + decode
  dist, softmax; gather winning kb rows (fp8) and accumulate
  diag(w) @ rows in PSUM with DoubleRow pairs; silu epilogue with the
  softmax normalization folded into the T-section scale.
"""

import math
import numpy as np

import concourse.bass as bass
import concourse.mybir as mybir
from concourse.bass import IndirectOffsetOnAxis
from concourse.tile import TileContext

AF = mybir.ActivationFunctionType
ALU = mybir.AluOpType
PM = mybir.MatmulPerfMode
dt = mybir.dt

N_CORES = 8
ALPHA_C, ALPHA_K, ALPHA_T = 1.0, 0.5, 0.25

# Ablation switches (bench only; production leaves these empty).
OPTS = {}


class Cfg:
    def __init__(self, rows=512, d=1024, nkb=65536, topk=32, temperature=1.0):
        assert rows % 128 == 0 and d % 256 == 0 and nkb % 1024 == 0
        assert topk % 8 == 0
        self.rows = rows          # batch rows per core
        self.d = d                # feature dim
        self.nkb = nkb            # knowledge-base rows
        self.topk = topk
        self.temp = float(temperature)
        self.RT = rows // 128     # row tiles
        self.CKN = d // 128       # contraction subtiles
        self.MB = 1024            # kb columns per chunk
        self.MC = nkb // self.MB  # kb chunks
        self.MC2 = self.MC // 2   # chunk pairs (selection granularity 2048)
        self.CAND = self.MC2 * 8  # candidate slots per row
        assert self.CAND >= topk
        assert self.MC <= 64 and self.MC % 2 == 0 and self.MB == 1024

        # u = 2*q.kb + (d - kb_sq) ~ N(0, sqrt(6d)). Map the top-k window
        # [u_lo, u_hi] onto fp16 integers [1024, 2046] (step exactly 1).
        def inv_q(p):  # inverse normal CDF via bisection on erfc
            lo_, hi_ = 0.0, 9.0
            for _ in range(80):
                m = 0.5 * (lo_ + hi_)
                if 0.5 * math.erfc(m / math.sqrt(2.0)) > p:
                    lo_ = m
                else:
                    hi_ = m
            return 0.5 * (lo_ + hi_)

        sigma = math.sqrt(6.0 * d)
        zthr = inv_q(topk / nkb)
        zmax = inv_q(1.0 / (2.0 * nkb))
        self.u_lo = (zthr - 0.35) * sigma
        self.u_hi = (zmax + 0.80) * sigma
        self.s = 1022.0 / (self.u_hi - self.u_lo)
        self.B = 1024.0 - self.s * self.u_lo


def build_body(tc, io, cfg: Cfg, ctx):
    """Emit the per-core program. io maps tensor names to DRAM APs."""
    nc = tc.nc
    RT, CKN, MB, MC, CAND, D = cfg.RT, cfg.CKN, cfg.MB, cfg.MC, cfg.CAND, cfg.d
    TOPK = cfg.topk
    NP = CKN // 2             # DoubleRow contraction pair count
    f8, f16, f32, u16, u32 = dt.float8e4, dt.float16, dt.float32, dt.uint16, dt.uint32

    ct, qt2w, kb4, nkb2 = io["ct"], io["qt2w"], io["kb4"], io["nkb2"]
    kbrows8, cbase_d, biasw_d = io["kbrows8"], io["cbase"], io["biasw"]
    eye_d, crows, krows, out = io["eye"], io["crows"], io["krows"], io["out"]

    const_pool = ctx.enter_context(tc.tile_pool(name="const", bufs=1))
    ones_col = const_pool.tile([128, 1], f16, tag="ones_col")
    nc.vector.memset(ones_col[:], 1.0)
    cbase_sb = const_pool.tile([128, CAND], f32, tag="cbase")
    nc.sync.dma_start(cbase_sb[:], cbase_d)
    biasw_sb = const_pool.tile([1, 2, 128], f8, tag="biasw")
    nc.sync.dma_start(biasw_sb[:], biasw_d)
    eye_sb = const_pool.tile([128, 128], f16, tag="eye")
    nc.sync.dma_start(eye_sb[:], eye_d)
    bB_sb = const_pool.tile([128, 1], f32, tag="bB")
    nc.vector.memset(bB_sb[:], cfg.B)

    persist = ctx.enter_context(tc.tile_pool(name="persist", bufs=1))
    qt_sb = persist.tile([128, CKN, cfg.rows], f8, tag="qt")
    b_sb = persist.tile([128, RT], f32, tag="bias")
    cand = persist.tile([128, RT, CAND], f16, tag="cand")
    cpos = persist.tile([128, RT, CAND], u16, tag="cpos")

    if OPTS.get("tiny"):
        with tc.tile_pool(name="o_pool", bufs=1) as o_pool:
            osb = o_pool.tile([128, 3 * D], f32, tag="osb")
            nc.vector.memset(osb[:], 0.0)
            for t in range(RT):
                nc.sync.dma_start(out[t * 128 : (t + 1) * 128, :], osb[:])
        return

    # ---- Phase 0: qT = (2*Q @ C.T) in fp16 -> fp8, plus per-row bias
    # b = q_sq + d - (1026 - B)/s  so that dist^2 = b - vq/s, where vq is
    # the winner's quad-masked window value (v & ~3) - 1024.
    bias_const = float(cfg.d - (1026.0 - cfg.B) / cfg.s)
    with (
        tc.tile_pool(name="p0_sbuf", bufs=2) as p0_pool,
        tc.tile_pool(name="p0_psum", bufs=2, space="PSUM") as p0_psum,
        tc.tile_pool(name="p0_qsq", bufs=1, space="PSUM") as p0_qsq,
    ):
        qt2w_sb = p0_pool.tile([128, CKN, cfg.d], f16, tag="qt2w")
        nc.sync.dma_start(qt2w_sb[:], qt2w)
        ct_sb = p0_pool.tile([128, CKN, cfg.rows], f16, tag="ct")
        nc.sync.dma_start(ct_sb[:], ct)

        qsq_ps = [
            p0_qsq.tile([128, 1], f32, name=f"qsq{t}", tag=f"qsq{t}")
            for t in range(RT)
        ]
        for j in range(CKN):
            qp = p0_psum.tile([128, cfg.rows], f32, tag="qproj")
            for ck in range(CKN):
                nc.tensor.matmul(
                    qp[:],
                    qt2w_sb[:, ck, j * 128 : (j + 1) * 128],
                    ct_sb[:, ck, :],
                    start=(ck == 0),
                    stop=(ck == CKN - 1),
                )
            nc.scalar.copy(qt_sb[:, j, :], qp[:])  # f32 -> fp8 on ACT
            sq = p0_pool.tile([128, cfg.rows], f16, tag="sq")
            nc.scalar.activation(sq[:], qp[:], AF.Square)
            for t in range(RT):
                nc.tensor.matmul(
                    qsq_ps[t][:],
                    sq[:, t * 128 : (t + 1) * 128],
                    ones_col[:],
                    start=(j == 0),
                    stop=(j == CKN - 1),
                )
        for t in range(RT):
            # qsq_ps holds sum((2q)^2) = 4*q_sq
            nc.scalar.activation(
                b_sb[:, t : t + 1], qsq_ps[t][:], AF.Copy, scale=0.25, bias=bias_const
            )

    # ---- Phase 1: stream kb chunk pairs; u = 2*q.kb + bias; quad-max top-8
    # per 2048 columns. fp16 sort keys carry a 2-bit quad id in the LSBs.
    with (
        tc.tile_pool(name="kb_pool", bufs=6) as kb_pool,
        tc.tile_pool(name="nb_pool", bufs=6) as nb_pool,
        tc.tile_pool(name="u_psum", bufs=3, space="PSUM") as u_psum,
        tc.tile_pool(name="v_pool", bufs=6) as v_pool,
        tc.tile_pool(name="k_pool", bufs=3) as k_pool,
        tc.tile_pool(name="m_pool", bufs=3) as m_pool,
        tc.tile_pool(name="mf_pool", bufs=4) as mf_pool,
    ):
        for c2 in range(MC // 2):
            kb_ts, nb_ts = [], []
            for ci in range(2):
                kb_t = kb_pool.tile([128, CKN, MB], f8, tag="kb")
                nc.sync.dma_start(kb_t[:], kb4[2 * c2 + ci])
                kb_ts.append(kb_t)
                nb_t = nb_pool.tile([1, 2, MB], f8, tag="nb")
                nc.sync.dma_start(nb_t[:], nkb2[2 * c2 + ci])
                nb_ts.append(nb_t)
            for t in range(RT):
                # vt[ci][quad][r]: fp16 sort keys for both chunks
                vt = v_pool.tile([128, 2, 4, 256], f16, tag="v")
                for ci in range(2):
                    ups = u_psum.tile([128, MB], f32, tag="u")
                    nobias = bool(OPTS.get("nobias"))
                    for h in range(2):
                        hs = ups[:, h * 512 : (h + 1) * 512]
                        for j in range(NP):
                            nc.tensor.matmul(
                                hs,
                                qt_sb[:, 2 * j : 2 * j + 2, t * 128 : (t + 1) * 128],
                                kb_ts[ci][:, 2 * j : 2 * j + 2, h * 512 : (h + 1) * 512],
                                start=(j == 0),
                                stop=(nobias and j == NP - 1),
                                perf_mode=PM.DoubleRow,
                            )
                        if not nobias:
                            nc.tensor.matmul(
                                hs,
                                biasw_sb[:],
                                nb_ts[ci][:, :, h * 512 : (h + 1) * 512],
                                start=False,
                                stop=True,
                                perf_mode=PM.DoubleRow,
                            )
                    nc.scalar.activation(
                        vt[:, ci], ups[:], AF.Relu, scale=cfg.s, bias=bB_sb[:]
                    )
                if OPTS.get("dbg") and c2 == 0 and t == 0:
                    nc.sync.dma_start(io["dbg_v"], vt[:])
                if OPTS.get("skip_topk"):
                    continue
                vu = vt[:].bitcast(u16)
                kt = k_pool.tile([128, 2, 4, 256], u16, tag="k")
                for qd in range(4):
                    nc.vector.tensor_scalar(
                        kt[:, :, qd, :],
                        vu[:, :, qd, :],
                        0x7FFC,
                        qd,
                        op0=ALU.bitwise_and,
                        op1=ALU.bitwise_or,
                    )
                mt = m_pool.tile([128, 2, 2, 256], u16, tag="m")
                nc.vector.tensor_tensor(
                    mt[:, :, 0, :], kt[:, :, 0, :], kt[:, :, 1, :], ALU.max
                )
                nc.vector.tensor_tensor(
                    mt[:, :, 1, :], kt[:, :, 2, :], kt[:, :, 3, :], ALU.max
                )
                mf = mf_pool.tile([128, 512], u16, tag="mf")
                nc.vector.tensor_tensor(
                    mf[:].rearrange("p (a b) -> p a b", a=2, b=256),
                    mt[:, :, 0, :],
                    mt[:, :, 1, :],
                    ALU.max,
                )
                mff = mf[:].bitcast(f16)
                cslice = cand[:, t, c2 * 8 : c2 * 8 + 8]
                nc.vector.max(out=cslice, in_=mff)
                nc.vector.max_index(
                    out=cpos[:, t, c2 * 8 : c2 * 8 + 8],
                    in_max=cslice,
                    in_values=mff,
                )
        if OPTS.get("skip_topk"):
            nc.vector.memset(cand[:], 1024.0)
            nc.vector.memset(cpos[:], 0)

    # ---- Phase 2: pack candidates, merge top-32, gather, PE reduce, epilogue
    with (
        tc.tile_pool(name="p2", bufs=2) as p2,
        tc.tile_pool(name="p2w", bufs=2) as p2w,
        tc.tile_pool(name="t_psum", bufs=4, space="PSUM") as t_psum,
        tc.tile_pool(name="g_pool", bufs=2) as g_pool,
        tc.tile_pool(name="dg_pool", bufs=6) as dg_pool,
        tc.tile_pool(name="o_pool", bufs=2) as o_pool,
    ):
        for t in range(RT):
            # pack = (v - 1024)*16384 + c*256 + pos  (exact ints < 2^24)
            cv = p2.tile([128, CAND], f32, tag="cv")
            nc.vector.tensor_copy(cv[:], cand[:, t, :])  # f16 -> f32
            px = p2.tile([128, CAND], f32, tag="px")
            nc.vector.tensor_copy(px[:], cpos[:, t, :])  # u16 -> f32
            nc.vector.tensor_add(px[:], px[:], cbase_sb[:])
            nc.vector.tensor_scalar(
                cv[:], cv[:], -1024.0, 16384.0, op0=ALU.add, op1=ALU.mult
            )
            nc.vector.tensor_add(cv[:], cv[:], px[:])

            # merge: rounds of top-8 extract + zap
            wv = p2w.tile([128, TOPK], f32, tag="wv")
            for r in range(TOPK // 8):
                nc.vector.max(out=wv[:, r * 8 : r * 8 + 8], in_=cv[:])
                if r < TOPK // 8 - 1:
                    nc.vector.match_replace(
                        out=cv[:],
                        in_to_replace=wv[:, r * 8 : r * 8 + 8],
                        in_values=cv[:],
                        imm_value=-1.0,
                    )
            # unpack: i = (v-1024)<<14 | c2<<9 | half<<8 | pos
            # gidx = c2*2048 + half*1024 + quad*256 + pos
            pku = p2w.tile([128, TOPK], u32, tag="pku")
            nc.vector.tensor_copy(pku[:], wv[:])  # exact: integer-valued f32
            ga = p2w.tile([128, TOPK], u32, tag="ga")
            nc.vector.tensor_scalar(
                ga[:], pku[:], 9, 31, op0=ALU.logical_shift_right, op1=ALU.bitwise_and
            )
            nc.vector.tensor_scalar(
                ga[:], ga[:], 11, None, op0=ALU.logical_shift_left
            )
            gh = p2w.tile([128, TOPK], u32, tag="gh")
            nc.vector.tensor_scalar(
                gh[:], pku[:], 256, 2, op0=ALU.bitwise_and,
                op1=ALU.logical_shift_left,
            )
            gb = p2w.tile([128, TOPK], u32, tag="gb")
            nc.vector.tensor_scalar(
                gb[:], pku[:], 6, 0x300, op0=ALU.logical_shift_right,
                op1=ALU.bitwise_and,
            )
            # gb = quad*256: quad sits at bits 14..15 of i, shifted to 8..9
            gidx = p2w.tile([128, TOPK], u32, tag="gidx")
            nc.vector.tensor_tensor(gidx[:], ga[:], gb[:], ALU.add)
            nc.vector.tensor_tensor(gidx[:], gidx[:], gh[:], ALU.add)
            gp = p2w.tile([128, TOPK], u32, tag="gp")
            nc.vector.tensor_scalar(
                gp[:], pku[:], 255, None, op0=ALU.bitwise_and
            )
            nc.vector.tensor_tensor(gidx[:], gidx[:], gp[:], ALU.add)
            # clamp (sentinel/junk rows only; full-size gidx is in-bounds)
            nc.vector.tensor_scalar(
                gidx[:], gidx[:], cfg.nkb - 1, None, op0=ALU.min
            )
            # vq = (i>>14) & 0xFFC : quad-masked window value - 1024
            vq = p2w.tile([128, TOPK], u32, tag="vq")
            nc.vector.tensor_scalar(
                vq[:], pku[:], 14, 0xFFC, op0=ALU.logical_shift_right,
                op1=ALU.bitwise_and,
            )
            vqf = p2w.tile([128, TOPK], f32, tag="vqf")
            nc.vector.tensor_copy(vqf[:], vq[:])
            # dist = sqrt(b - vq/s)
            dist = p2w.tile([128, TOPK], f32, tag="dist")
            nc.scalar.activation(
                dist[:],
                vqf[:],
                AF.Sqrt,
                scale=-1.0 / cfg.s,
                bias=b_sb[:, t : t + 1],
            )
            # unnormalized softmax over -dist/temp; normalization folds into
            # the final silu's per-row T scale.
            dmin = p2w.tile([128, 1], f32, tag="dmin")
            nc.vector.tensor_reduce(dmin[:], dist[:], mybir.AxisListType.X, ALU.min)
            nc.vector.tensor_scalar_mul(dmin[:], dmin[:], 1.0 / cfg.temp)
            ex = p2w.tile([128, TOPK], f32, tag="ex")
            se = p2w.tile([128, 1], f32, tag="se")
            nc.scalar.activation(
                ex[:], dist[:], AF.Exp, scale=-1.0 / cfg.temp, bias=dmin[:],
                accum_out=se[:],
            )
            fs = p2w.tile([128, 1], f32, tag="fs")
            nc.vector.reciprocal(fs[:], se[:])
            nc.vector.tensor_scalar_mul(fs[:], fs[:], ALPHA_T)

            # gather all winners (fp8) in one SWDGE batch, then accumulate
            # diag(ex_k) @ rows on the PE
            gk = g_pool.tile([128, TOPK, D], f8, tag="gk")
            if OPTS.get("nogather"):
                nc.vector.memset(gk[:], 0.25)
            else:
                # multi-index offset APs hang the HW SWDGE path; one
                # [128,1]-offset gather per winner is the supported form.
                for gi in range(TOPK):
                    nc.gpsimd.indirect_dma_start(
                        gk[:, gi, :],
                        None,
                        kbrows8,
                        IndirectOffsetOnAxis(ap=gidx[:, gi : gi + 1], axis=0),
                    )
            tpsA = t_psum.tile([128, 512], f32, tag="tpsA")
            tpsB = t_psum.tile([128, 512], f32, tag="tpsB")
            NPAIR = TOPK // 2
            for p in range(NPAIR):
                dk = dg_pool.tile([128, 2, 128], f8, tag="dk")
                for j in range(2):
                    nc.scalar.activation(
                        dk[:, j, :], eye_sb[:], AF.Copy,
                        scale=ex[:, 2 * p + j : 2 * p + j + 1],
                    )
                nc.tensor.matmul(
                    tpsA[:], dk[:], gk[:, 2 * p : 2 * p + 2, 0:512],
                    start=(p == 0), stop=(p == NPAIR - 1), perf_mode=PM.DoubleRow,
                )
                nc.tensor.matmul(
                    tpsB[:], dk[:], gk[:, 2 * p : 2 * p + 2, 512:1024],
                    start=(p == 0), stop=(p == NPAIR - 1), perf_mode=PM.DoubleRow,
                )

            if OPTS.get("dbg") and t == 0:
                nc.sync.dma_start(io["dbg_cand"], cand[:, 0, :])
                nc.sync.dma_start(io["dbg_cpos"], cpos[:, 0, :])
                nc.sync.dma_start(io["dbg_wv"], wv[:])
                nc.sync.dma_start(io["dbg_gidx"], gidx[:])
                nc.sync.dma_start(io["dbg_dist"], dist[:])

            # epilogue: out = silu([aC*C, aK*K, aT*T]), T scale = aT/se
            SILU = AF.Copy if OPTS.get("nosilu") else AF.Silu
            osb = o_pool.tile([128, 3 * D], f32, tag="osb")
            cl = o_pool.tile([128, D], f32, tag="cl")
            nc.sync.dma_start(cl[:], crows[t * 128 : (t + 1) * 128, :])
            nc.scalar.activation(osb[:, 0:D], cl[:], SILU, scale=ALPHA_C)
            kl = o_pool.tile([128, D], f32, tag="kl")
            nc.sync.dma_start(kl[:], krows[t * 128 : (t + 1) * 128, :])
            nc.scalar.activation(osb[:, D : 2 * D], kl[:], SILU, scale=ALPHA_K)
            nc.scalar.activation(
                osb[:, 2 * D : 2 * D + 512], tpsA[:], SILU, scale=fs[:]
            )
            nc.scalar.activation(
                osb[:, 2 * D + 512 : 3 * D], tpsB[:], SILU, scale=fs[:]
            )
            nc.sync.dma_start(out[t * 128 : (t + 1) * 128, :], osb[:])


def split_sync_waits(nc, limit=1):
    """This walrus build rejects instructions with >1 semaphore wait; move
    excess waits onto InstNoOp carriers inserted just before."""
    n_split = 0
    for bb in nc.m.functions[0].blocks:
        insts = list(bb.instructions)
        out = []
        changed = False
        for inst in insts:
            si = inst.sync_info
            waits = list(si.on_wait) if si is not None else []
            if len(waits) > limit:
                extra, keep = waits[:-limit], waits[-limit:]
                chunks = [extra[i : i + limit] for i in range(0, len(extra), limit)]
                for j, ch in enumerate(chunks):
                    out.append(
                        mybir.InstNoOp(
                            name=f"{inst.name}-wsplit{j}",
                            text_hint="wait_split",
                            bass_nofuse=True,
                            engine=inst.engine,
                            sync_info=mybir.SyncInfo(on_wait=ch, on_update=[]),
                        )
                    )
                inst.sync_info = mybir.SyncInfo(
                    on_wait=keep, on_update=list(si.on_update)
                )
                n_split += 1
                changed = True
            out.append(inst)
        if changed:
            bb.instructions = out
    return n_split


def build_nc(cfg: Cfg, for_sim=False):
    from contextlib import ExitStack

    nc = bass.Bass("TRN2", target_bir_lowering=False, debug=False)
    f8, f16, f32 = dt.float8e4, dt.float16, dt.float32
    io = {
        "ct": nc.dram_tensor("ct", [128, cfg.CKN, cfg.rows], f16, kind="ExternalInput").ap(),
        "qt2w": nc.dram_tensor("qt2w", [128, cfg.CKN, cfg.d], f16, kind="ExternalInput").ap(),
        "kb4": nc.dram_tensor("kb4", [cfg.MC, 128, cfg.CKN, cfg.MB], f8, kind="ExternalInput").ap(),
        "nkb2": nc.dram_tensor("nkb2", [cfg.MC, 1, 2, cfg.MB], f8, kind="ExternalInput").ap(),
        "kbrows8": nc.dram_tensor("kbrows8", [cfg.nkb, cfg.d], f8, kind="ExternalInput").ap(),
        "cbase": nc.dram_tensor("cbase", [128, cfg.CAND], f32, kind="ExternalInput").ap(),
        "biasw": nc.dram_tensor("biasw", [1, 2, 128], f8, kind="ExternalInput").ap(),
        "eye": nc.dram_tensor("eye", [128, 128], f16, kind="ExternalInput").ap(),
        "crows": nc.dram_tensor("crows", [cfg.rows, cfg.d], f32, kind="ExternalInput").ap(),
        "krows": nc.dram_tensor("krows", [cfg.rows, cfg.d], f32, kind="ExternalInput").ap(),
        "out": nc.dram_tensor("out", [cfg.rows, 3 * cfg.d], f32, kind="ExternalOutput").ap(),
    }
    if OPTS.get("dbg"):
        u16_, u32_ = dt.uint16, dt.uint32
        io["dbg_cand"] = nc.dram_tensor("dbg_cand", [128, cfg.CAND], f16, kind="ExternalOutput").ap()
        io["dbg_cpos"] = nc.dram_tensor("dbg_cpos", [128, cfg.CAND], u16_, kind="ExternalOutput").ap()
        io["dbg_wv"] = nc.dram_tensor("dbg_wv", [128, cfg.topk], f32, kind="ExternalOutput").ap()
        io["dbg_gidx"] = nc.dram_tensor("dbg_gidx", [128, cfg.topk], u32_, kind="ExternalOutput").ap()
        io["dbg_dist"] = nc.dram_tensor("dbg_dist", [128, cfg.topk], f32, kind="ExternalOutput").ap()
        io["dbg_v"] = nc.dram_tensor("dbg_v", [128, 2, 4, 256], f16, kind="ExternalOutput").ap()
    with TileContext(nc) as tc:
        with ExitStack() as ctx:
            build_body(tc, io, cfg, ctx)
    if not for_sim:
        split_sync_waits(nc, limit=1)
    return nc


def host_prep(C, K, tie_kb, Q_weight, cfg: Cfg, n_cores=N_CORES):
    """Build per-core input maps from full inputs."""
    import ml_dtypes

    F8 = ml_dtypes.float8_e4m3
    C = np.asarray(C, dtype=np.float32)
    K = np.asarray(K, dtype=np.float32)
    tie_kb = np.asarray(tie_kb, dtype=np.float32)
    Q_weight = np.asarray(Q_weight, dtype=np.float32)
    d, rows, MC, MB, CKN = cfg.d, cfg.rows, cfg.MC, cfg.MB, cfg.CKN

    kb8 = tie_kb.astype(F8)
    # kb4[c, p, ck, m] = kb8[c*MB+m, ck*128+p]
    kb4 = np.ascontiguousarray(
        kb8.reshape(MC, MB, CKN, 128).transpose(0, 3, 2, 1)
    )
    kbsq = (tie_kb.astype(np.float32) ** 2).sum(axis=1)
    vbias = np.float32(d) - kbsq.astype(np.float32)
    hi = np.round(vbias / 32.0).astype(F8)
    lo = (vbias - 32.0 * hi.astype(np.float32)).astype(F8)
    nkb2 = np.stack([hi, lo], axis=0).reshape(2, MC, MB).transpose(1, 0, 2)
    nkb2 = np.ascontiguousarray(nkb2.reshape(MC, 1, 2, MB))
    biasw = np.empty((1, 2, 128), dtype=F8)
    biasw[0, 0, :] = F8(32.0)
    biasw[0, 1, :] = F8(1.0)
    # qt2w[p, ck, c'] = 2*Q_weight[c', ck*128+p]
    qt2w = np.ascontiguousarray(
        (2.0 * Q_weight.T).astype(np.float16).reshape(CKN, 128, d).transpose(1, 0, 2)
    )
    cbase = np.broadcast_to(
        ((np.arange(cfg.CAND, dtype=np.float32) // 8) * 512.0), (128, cfg.CAND)
    ).copy()
    eye = np.eye(128, dtype=np.float16)

    in_maps = []
    for i in range(n_cores):
        Cs = C[i * rows : (i + 1) * rows]
        Ks = K[i * rows : (i + 1) * rows]
        ct = np.ascontiguousarray(
            Cs.T.astype(np.float16).reshape(CKN, 128, rows).transpose(1, 0, 2)
        )
        in_maps.append(
            {
                "ct": ct,
                "qt2w": qt2w,
                "kb4": kb4,
                "nkb2": nkb2,
                "kbrows8": kb8,
                "cbase": cbase,
                "biasw": biasw,
                "eye": eye,
                "crows": np.ascontiguousarray(Cs),
                "krows": np.ascontiguousarray(Ks),
            }
        )
    return in_maps


_NC_CACHE = {}


def kernel(C, K, tie_kb, Q_weight, temperature=1.0, top_k=32):
    from concourse.bass_utils import run_bass_kernel_spmd

    C = np.asarray(C)
    n = C.shape[0]
    cfg = Cfg(
        rows=n // N_CORES,
        d=C.shape[1],
        nkb=np.asarray(tie_kb).shape[0],
        topk=int(top_k),
        temperature=float(temperature),
    )
    key = (cfg.rows, cfg.d, cfg.nkb, cfg.topk, cfg.temp)
    if key not in _NC_CACHE:
        _NC_CACHE[key] = build_nc(cfg)
    nc = _NC_CACHE[key]
    in_maps = host_prep(C, K, tie_kb, Q_weight, cfg)
    res = run_bass_kernel_spmd(nc, in_maps, core_ids=list(range(N_CORES)))
    return np.concatenate([res.results[i]["out"] for i in range(N_CORES)], axis=0)


# revision 39
# speedup vs baseline: 1.2667x; 1.2667x over previous
"""Trainium2 Bass kernel for nn_CVLFuser (retrieval KNN fuser), v2.

out = silu(concat([1.0*C, 0.5*K, 0.25*T], axis=1)) where T is the
softmax(-cdist/temp)-weighted sum of the top_k nearest tie_kb rows to
q = C @ Q_weight.T.

Sharding: data-parallel over the batch dim across 8 NeuronCores; tie_kb
replicated. Per core (512 rows x 65536 KB):

Phase 1 (per 1024-col kb chunk x 128-row tile):
  - u = 2q.kb + (d - kb_sq) via fp8 DoubleRow matmuls; the (d - kb_sq)
    bias rides as one extra DoubleRow pair with fp8 planes (32*hi + lo).
  - One ACT pass evacuates the [128,1024] PSUM as fp16
    relu(s*u + B) -- integer-valued in [1024, 2046] (10-bit quantizer).
  - DVE steals the 2 value LSBs as a quad-id (tensor_scalar and/or at
    4x), reduces 4 quads with a tensor_tensor max tree (2x), then runs
    Max/MaxIndex on just [128,256]: top-8 per chunk with the full index
    recoverable as c*1024 + quad*256 + pos.
Phase 2 (per row tile): pack candidates as
  (v-1024)*16384 + c*256 + pos  (exact f32 ints < 2^24), merge top-32
  via Max/match_replace rounds, unpack bits to global indices + decode
  dist, softmax; gather winning kb rows (fp8) and accumulate
  diag(w) @ rows in PSUM with DoubleRow pairs; silu epilogue with the
  softmax normalization folded into the T-section scale.
"""

import math
import numpy as np

import concourse.bass as bass
import concourse.mybir as mybir
from concourse.bass import IndirectOffsetOnAxis
from concourse.tile import TileContext

AF = mybir.ActivationFunctionType
ALU = mybir.AluOpType
PM = mybir.MatmulPerfMode
dt = mybir.dt

N_CORES = 8
ALPHA_C, ALPHA_K, ALPHA_T = 1.0, 0.5, 0.25

# Ablation switches (bench only; production leaves these empty).
OPTS = {}


class Cfg:
    def __init__(self, rows=512, d=1024, nkb=65536, topk=32, temperature=1.0):
        assert rows % 128 == 0 and d % 256 == 0 and nkb % 1024 == 0
        assert topk % 8 == 0
        self.rows = rows          # batch rows per core
        self.d = d                # feature dim
        self.nkb = nkb            # knowledge-base rows
        self.topk = topk
        self.temp = float(temperature)
        self.RT = rows // 128     # row tiles
        self.CKN = d // 128       # contraction subtiles
        self.MB = 1024            # kb columns per chunk
        self.MC = nkb // self.MB  # kb chunks
        self.MC2 = self.MC // 2   # chunk pairs (selection granularity 2048)
        self.CAND = self.MC2 * 8  # candidate slots per row
        assert self.CAND >= topk
        assert self.MC <= 64 and self.MC % 2 == 0 and self.MB == 1024

        # u = 2*q.kb + (d - kb_sq) ~ N(0, sqrt(6d)). Map the top-k window
        # [u_lo, u_hi] onto fp16 integers [1024, 2046] (step exactly 1).
        def inv_q(p):  # inverse normal CDF via bisection on erfc
            lo_, hi_ = 0.0, 9.0
            for _ in range(80):
                m = 0.5 * (lo_ + hi_)
                if 0.5 * math.erfc(m / math.sqrt(2.0)) > p:
                    lo_ = m
                else:
                    hi_ = m
            return 0.5 * (lo_ + hi_)

        sigma = math.sqrt(6.0 * d)
        zthr = inv_q(topk / nkb)
        zmax = inv_q(1.0 / (2.0 * nkb))
        self.u_lo = (zthr - 0.35) * sigma
        self.u_hi = (zmax + 0.80) * sigma
        self.s = 1022.0 / (self.u_hi - self.u_lo)
        self.B = 1024.0 - self.s * self.u_lo


def build_body(tc, io, cfg: Cfg, ctx):
    """Emit the per-core program. io maps tensor names to DRAM APs."""
    nc = tc.nc
    RT, CKN, MB, MC, CAND, D = cfg.RT, cfg.CKN, cfg.MB, cfg.MC, cfg.CAND, cfg.d
    TOPK = cfg.topk
    NP = CKN // 2             # DoubleRow contraction pair count
    f8, f16, f32, u16, u32 = dt.float8e4, dt.float16, dt.float32, dt.uint16, dt.uint32

    ct, qt2w, kb4, nkb2 = io["ct"], io["qt2w"], io["kb4"], io["nkb2"]
    kbrows8, cbase_d, biasw_d = io["kbrows8"], io["cbase"], io["biasw"]
    eye_d, crows, krows, out = io["eye"], io["crows"], io["krows"], io["out"]

    const_pool = ctx.enter_context(tc.tile_pool(name="const", bufs=1))
    ones_col = const_pool.tile([128, 1], f16, tag="ones_col")
    nc.vector.memset(ones_col[:], 1.0)
    cbase_sb = const_pool.tile([128, CAND], f32, tag="cbase")
    nc.sync.dma_start(cbase_sb[:], cbase_d)
    biasw_sb = const_pool.tile([1, 2, 128], f8, tag="biasw")
    nc.sync.dma_start(biasw_sb[:], biasw_d)
    eye_sb = const_pool.tile([128, 128], f16, tag="eye")
    nc.sync.dma_start(eye_sb[:], eye_d)
    bB_sb = const_pool.tile([128, 1], f32, tag="bB")
    nc.vector.memset(bB_sb[:], cfg.B)

    persist = ctx.enter_context(tc.tile_pool(name="persist", bufs=1))
    qt_sb = persist.tile([128, CKN, cfg.rows], f8, tag="qt")
    b_sb = persist.tile([128, RT], f32, tag="bias")
    cand = persist.tile([128, RT, CAND], f16, tag="cand")
    cpos = persist.tile([128, RT, CAND], u16, tag="cpos")

    if OPTS.get("tiny"):
        with tc.tile_pool(name="o_pool", bufs=1) as o_pool:
            osb = o_pool.tile([128, 3 * D], f32, tag="osb")
            nc.vector.memset(osb[:], 0.0)
            for t in range(RT):
                nc.sync.dma_start(out[t * 128 : (t + 1) * 128, :], osb[:])
        return

    if OPTS.get("dmaonly"):
        with (
            tc.tile_pool(name="kb_pool", bufs=6) as kb_pool,
            tc.tile_pool(name="o_pool", bufs=1) as o_pool,
        ):
            for c in range(MC):
                kb_t = kb_pool.tile([128, CKN, MB], f8, tag="kb")
                nc.sync.dma_start(kb_t[:], kb4[c])
                nc.vector.tensor_copy(cand[:, 0, 0:8], kb_t[:, 0, 0:16].bitcast(f16))
            osb = o_pool.tile([128, 3 * D], f32, tag="osb")
            nc.vector.memset(osb[:], 0.0)
            for t in range(RT):
                nc.sync.dma_start(out[t * 128 : (t + 1) * 128, :], osb[:])
        return

    # ---- Phase 0: qT = (2*Q @ C.T) in fp16 -> fp8, plus per-row bias
    # b = q_sq + d - (1026 - B)/s  so that dist^2 = b - vq/s, where vq is
    # the winner's quad-masked window value (v & ~3) - 1024.
    bias_const = float(cfg.d - (1026.0 - cfg.B) / cfg.s)
    with (
        tc.tile_pool(name="p0_sbuf", bufs=2) as p0_pool,
        tc.tile_pool(name="p0_psum", bufs=2, space="PSUM") as p0_psum,
        tc.tile_pool(name="p0_qsq", bufs=1, space="PSUM") as p0_qsq,
    ):
        qt2w_sb = p0_pool.tile([128, CKN, cfg.d], f16, tag="qt2w")
        nc.sync.dma_start(qt2w_sb[:], qt2w)
        ct_sb = p0_pool.tile([128, CKN, cfg.rows], f16, tag="ct")
        nc.sync.dma_start(ct_sb[:], ct)

        qsq_ps = [
            p0_qsq.tile([128, 1], f32, name=f"qsq{t}", tag=f"qsq{t}")
            for t in range(RT)
        ]
        for j in range(CKN):
            qp = p0_psum.tile([128, cfg.rows], f32, tag="qproj")
            for ck in range(CKN):
                nc.tensor.matmul(
                    qp[:],
                    qt2w_sb[:, ck, j * 128 : (j + 1) * 128],
                    ct_sb[:, ck, :],
                    start=(ck == 0),
                    stop=(ck == CKN - 1),
                )
            nc.scalar.copy(qt_sb[:, j, :], qp[:])  # f32 -> fp8 on ACT
            sq = p0_pool.tile([128, cfg.rows], f16, tag="sq")
            nc.scalar.activation(sq[:], qp[:], AF.Square)
            for t in range(RT):
                nc.tensor.matmul(
                    qsq_ps[t][:],
                    sq[:, t * 128 : (t + 1) * 128],
                    ones_col[:],
                    start=(j == 0),
                    stop=(j == CKN - 1),
                )
        for t in range(RT):
            # qsq_ps holds sum((2q)^2) = 4*q_sq
            nc.scalar.activation(
                b_sb[:, t : t + 1], qsq_ps[t][:], AF.Copy, scale=0.25, bias=bias_const
            )

    # ---- Phase 1: stream kb chunk pairs; u = 2*q.kb + bias; quad-max top-8
    # per 2048 columns. fp16 sort keys carry a 2-bit quad id in the LSBs.
    with (
        tc.tile_pool(name="kb_pool", bufs=6) as kb_pool,
        tc.tile_pool(name="nb_pool", bufs=6) as nb_pool,
        tc.tile_pool(name="u_psum", bufs=4, space="PSUM") as u_psum,
        tc.tile_pool(name="v_pool", bufs=6) as v_pool,
        tc.tile_pool(name="k_pool", bufs=3) as k_pool,
        tc.tile_pool(name="m_pool", bufs=3) as m_pool,
        tc.tile_pool(name="mf_pool", bufs=4) as mf_pool,
    ):
        for c2 in range(MC // 2):
            kb_ts, nb_ts = [], []
            for ci in range(2):
                kb_t = kb_pool.tile([128, CKN, MB], f8, tag="kb")
                nc.sync.dma_start(kb_t[:], kb4[2 * c2 + ci])
                kb_ts.append(kb_t)
                nb_t = nb_pool.tile([1, 2, MB], f8, tag="nb")
                nc.sync.dma_start(nb_t[:], nkb2[2 * c2 + ci])
                nb_ts.append(nb_t)
            for t in range(RT):
                # vt[ci][quad][r]: fp16 sort keys for both chunks
                vt = v_pool.tile([128, 2, 4, 256], f16, tag="v")
                nobias = bool(OPTS.get("nobias"))
                NPe = NP // 2 if OPTS.get("halfmm") else NP
                # weights stationary across 4 streams (2 chunks x 2 halves)
                upss = [
                    u_psum.tile([128, MB], f32, name=f"u{ci}", tag="u")
                    for ci in range(2)
                ]
                for j in range(NPe):
                    wj = qt_sb[:, 2 * j : 2 * j + 2, t * 128 : (t + 1) * 128]
                    for ci in range(2):
                        for h in range(2):
                            nc.tensor.matmul(
                                upss[ci][:, h * 512 : (h + 1) * 512],
                                wj,
                                kb_ts[ci][:, 2 * j : 2 * j + 2, h * 512 : (h + 1) * 512],
                                start=(j == 0),
                                stop=(nobias and j == NPe - 1),
                                perf_mode=PM.DoubleRow,
                            )
                if not nobias:
                    for ci in range(2):
                        for h in range(2):
                            nc.tensor.matmul(
                                upss[ci][:, h * 512 : (h + 1) * 512],
                                biasw_sb[:],
                                nb_ts[ci][:, :, h * 512 : (h + 1) * 512],
                                start=False,
                                stop=True,
                                perf_mode=PM.DoubleRow,
                            )
                for ci in range(2):
                    if OPTS.get("skipevac"):
                        nc.scalar.copy(
                            cand[:, t, c2 * 8 : c2 * 8 + 8], upss[ci][:, 0:8]
                        )
                    else:
                        nc.scalar.activation(
                            vt[:, ci], upss[ci][:], AF.Relu, scale=cfg.s, bias=bB_sb[:]
                        )
                if OPTS.get("dbg") and c2 == 0 and t == 0:
                    nc.sync.dma_start(io["dbg_v"], vt[:])
                if OPTS.get("skip_topk"):
                    continue
                vu = vt[:].bitcast(u16)
                kt = k_pool.tile([128, 2, 4, 256], u16, tag="k")
                for qd in range(4):
                    nc.vector.tensor_scalar(
                        kt[:, :, qd, :],
                        vu[:, :, qd, :],
                        0x7FFC,
                        qd,
                        op0=ALU.bitwise_and,
                        op1=ALU.bitwise_or,
                    )
                mt = m_pool.tile([128, 2, 2, 256], u16, tag="m")
                nc.vector.tensor_tensor(
                    mt[:, :, 0, :], kt[:, :, 0, :], kt[:, :, 1, :], ALU.max
                )
                nc.vector.tensor_tensor(
                    mt[:, :, 1, :], kt[:, :, 2, :], kt[:, :, 3, :], ALU.max
                )
                mf = mf_pool.tile([128, 512], u16, tag="mf")
                nc.vector.tensor_tensor(
                    mf[:].rearrange("p (a b) -> p a b", a=2, b=256),
                    mt[:, :, 0, :],
                    mt[:, :, 1, :],
                    ALU.max,
                )
                mff = mf[:].bitcast(f16)
                cslice = cand[:, t, c2 * 8 : c2 * 8 + 8]
                nc.vector.max(out=cslice, in_=mff)
                nc.vector.max_index(
                    out=cpos[:, t, c2 * 8 : c2 * 8 + 8],
                    in_max=cslice,
                    in_values=mff,
                )
        if OPTS.get("skip_topk"):
            nc.vector.memset(cand[:], 1024.0)
            nc.vector.memset(cpos[:], 0)

    # ---- Phase 2: pack candidates, merge top-32, gather, PE reduce, epilogue
    with (
        tc.tile_pool(name="p2", bufs=2) as p2,
        tc.tile_pool(name="p2w", bufs=2) as p2w,
        tc.tile_pool(name="t_psum", bufs=4, space="PSUM") as t_psum,
        tc.tile_pool(name="g_pool", bufs=2) as g_pool,
        tc.tile_pool(name="dg_pool", bufs=6) as dg_pool,
        tc.tile_pool(name="o_pool", bufs=2) as o_pool,
    ):
        for t in range(RT):
            if OPTS.get("skip_phase2"):
                osb = o_pool.tile([128, 3 * D], f32, tag="osb")
                nc.vector.memset(osb[:], 0.0)
                nc.sync.dma_start(out[t * 128 : (t + 1) * 128, :], osb[:])
                continue
            # pack = (v - 1024)*16384 + c*256 + pos  (exact ints < 2^24)
            cv = p2.tile([128, CAND], f32, tag="cv")
            nc.vector.tensor_copy(cv[:], cand[:, t, :])  # f16 -> f32
            px = p2.tile([128, CAND], f32, tag="px")
            nc.vector.tensor_copy(px[:], cpos[:, t, :])  # u16 -> f32
            nc.vector.tensor_add(px[:], px[:], cbase_sb[:])
            nc.vector.tensor_scalar(
                cv[:], cv[:], -1024.0, 16384.0, op0=ALU.add, op1=ALU.mult
            )
            nc.vector.tensor_add(cv[:], cv[:], px[:])

            # merge: rounds of top-8 extract + zap
            wv = p2w.tile([128, TOPK], f32, tag="wv")
            for r in range(TOPK // 8):
                nc.vector.max(out=wv[:, r * 8 : r * 8 + 8], in_=cv[:])
                if r < TOPK // 8 - 1:
                    nc.vector.match_replace(
                        out=cv[:],
                        in_to_replace=wv[:, r * 8 : r * 8 + 8],
                        in_values=cv[:],
                        imm_value=-1.0,
                    )
            # unpack: i = (v-1024)<<14 | c2<<9 | half<<8 | pos
            # gidx = c2*2048 + half*1024 + quad*256 + pos
            pku = p2w.tile([128, TOPK], u32, tag="pku")
            nc.vector.tensor_copy(pku[:], wv[:])  # exact: integer-valued f32
            ga = p2w.tile([128, TOPK], u32, tag="ga")
            nc.vector.tensor_scalar(
                ga[:], pku[:], 9, 31, op0=ALU.logical_shift_right, op1=ALU.bitwise_and
            )
            nc.vector.tensor_scalar(
                ga[:], ga[:], 11, None, op0=ALU.logical_shift_left
            )
            gh = p2w.tile([128, TOPK], u32, tag="gh")
            nc.vector.tensor_scalar(
                gh[:], pku[:], 256, 2, op0=ALU.bitwise_and,
                op1=ALU.logical_shift_left,
            )
            gb = p2w.tile([128, TOPK], u32, tag="gb")
            nc.vector.tensor_scalar(
                gb[:], pku[:], 6, 0x300, op0=ALU.logical_shift_right,
                op1=ALU.bitwise_and,
            )
            # gb = quad*256: quad sits at bits 14..15 of i, shifted to 8..9
            gidx = p2w.tile([128, TOPK], u32, tag="gidx")
            nc.vector.tensor_tensor(gidx[:], ga[:], gb[:], ALU.add)
            nc.vector.tensor_tensor(gidx[:], gidx[:], gh[:], ALU.add)
            gp = p2w.tile([128, TOPK], u32, tag="gp")
            nc.vector.tensor_scalar(
                gp[:], pku[:], 255, None, op0=ALU.bitwise_and
            )
            nc.vector.tensor_tensor(gidx[:], gidx[:], gp[:], ALU.add)
            # clamp (sentinel/junk rows only; full-size gidx is in-bounds)
            nc.vector.tensor_scalar(
                gidx[:], gidx[:], cfg.nkb - 1, None, op0=ALU.min
            )
            # vq = (i>>14) & 0xFFC : quad-masked window value - 1024
            vq = p2w.tile([128, TOPK], u32, tag="vq")
            nc.vector.tensor_scalar(
                vq[:], pku[:], 14, 0xFFC, op0=ALU.logical_shift_right,
                op1=ALU.bitwise_and,
            )
            vqf = p2w.tile([128, TOPK], f32, tag="vqf")
            nc.vector.tensor_copy(vqf[:], vq[:])
            # dist = sqrt(b - vq/s)
            dist = p2w.tile([128, TOPK], f32, tag="dist")
            nc.scalar.activation(
                dist[:],
                vqf[:],
                AF.Sqrt,
                scale=-1.0 / cfg.s,
                bias=b_sb[:, t : t + 1],
            )
            # unnormalized softmax over -dist/temp; normalization folds into
            # the final silu's per-row T scale.
            dmin = p2w.tile([128, 1], f32, tag="dmin")
            nc.vector.tensor_reduce(dmin[:], dist[:], mybir.AxisListType.X, ALU.min)
            nc.vector.tensor_scalar_mul(dmin[:], dmin[:], 1.0 / cfg.temp)
            ex = p2w.tile([128, TOPK], f32, tag="ex")
            se = p2w.tile([128, 1], f32, tag="se")
            nc.scalar.activation(
                ex[:], dist[:], AF.Exp, scale=-1.0 / cfg.temp, bias=dmin[:],
                accum_out=se[:],
            )
            fs = p2w.tile([128, 1], f32, tag="fs")
            nc.vector.reciprocal(fs[:], se[:])
            nc.vector.tensor_scalar_mul(fs[:], fs[:], ALPHA_T)

            # gather all winners (fp8) in one SWDGE batch, then accumulate
            # diag(ex_k) @ rows on the PE
            gk = g_pool.tile([128, TOPK, D], f8, tag="gk")
            if OPTS.get("nogather"):
                nc.vector.memset(gk[:], 0.25)
            else:
                # multi-index offset APs hang the HW SWDGE path; one
                # [128,1]-offset gather per winner is the supported form.
                for gi in range(TOPK):
                    nc.gpsimd.indirect_dma_start(
                        gk[:, gi, :],
                        None,
                        kbrows8,
                        IndirectOffsetOnAxis(ap=gidx[:, gi : gi + 1], axis=0),
                    )
            tpsA = t_psum.tile([128, 512], f32, tag="tpsA")
            tpsB = t_psum.tile([128, 512], f32, tag="tpsB")
            NPAIR = TOPK // 2
            for p in range(NPAIR):
                dk = dg_pool.tile([128, 2, 128], f8, tag="dk")
                for j in range(2):
                    nc.scalar.activation(
                        dk[:, j, :], eye_sb[:], AF.Copy,
                        scale=ex[:, 2 * p + j : 2 * p + j + 1],
                    )
                nc.tensor.matmul(
                    tpsA[:], dk[:], gk[:, 2 * p : 2 * p + 2, 0:512],
                    start=(p == 0), stop=(p == NPAIR - 1), perf_mode=PM.DoubleRow,
                )
                nc.tensor.matmul(
                    tpsB[:], dk[:], gk[:, 2 * p : 2 * p + 2, 512:1024],
                    start=(p == 0), stop=(p == NPAIR - 1), perf_mode=PM.DoubleRow,
                )

            if OPTS.get("dbg") and t == 0:
                nc.sync.dma_start(io["dbg_cand"], cand[:, 0, :])
                nc.sync.dma_start(io["dbg_cpos"], cpos[:, 0, :])
                nc.sync.dma_start(io["dbg_wv"], wv[:])
                nc.sync.dma_start(io["dbg_gidx"], gidx[:])
                nc.sync.dma_start(io["dbg_dist"], dist[:])

            # epilogue: out = silu([aC*C, aK*K, aT*T]), T scale = aT/se
            SILU = AF.Copy if OPTS.get("nosilu") else AF.Silu
            osb = o_pool.tile([128, 3 * D], f32, tag="osb")
            cl = o_pool.tile([128, D], f32, tag="cl")
            nc.sync.dma_start(cl[:], crows[t * 128 : (t + 1) * 128, :])
            nc.scalar.activation(osb[:, 0:D], cl[:], SILU, scale=ALPHA_C)
            kl = o_pool.tile([128, D], f32, tag="kl")
            nc.sync.dma_start(kl[:], krows[t * 128 : (t + 1) * 128, :])
            nc.scalar.activation(osb[:, D : 2 * D], kl[:], SILU, scale=ALPHA_K)
            nc.scalar.activation(
                osb[:, 2 * D : 2 * D + 512], tpsA[:], SILU, scale=fs[:]
            )
            nc.scalar.activation(
                osb[:, 2 * D + 512 : 3 * D], tpsB[:], SILU, scale=fs[:]
            )
            nc.sync.dma_start(out[t * 128 : (t + 1) * 128, :], osb[:])


def split_sync_waits(nc, limit=1):
    """This walrus build rejects instructions with >1 semaphore wait; move
    excess waits onto InstNoOp carriers inserted just before."""
    n_split = 0
    for bb in nc.m.functions[0].blocks:
        insts = list(bb.instructions)
        out = []
        changed = False
        for inst in insts:
            si = inst.sync_info
            waits = list(si.on_wait) if si is not None else []
            if len(waits) > limit:
                extra, keep = waits[:-limit], waits[-limit:]
                chunks = [extra[i : i + limit] for i in range(0, len(extra), limit)]
                for j, ch in enumerate(chunks):
                    out.append(
                        mybir.InstNoOp(
                            name=f"{inst.name}-wsplit{j}",
                            text_hint="wait_split",
                            bass_nofuse=True,
                            engine=inst.engine,
                            sync_info=mybir.SyncInfo(on_wait=ch, on_update=[]),
                        )
                    )
                inst.sync_info = mybir.SyncInfo(
                    on_wait=keep, on_update=list(si.on_update)
                )
                n_split += 1
                changed = True
            out.append(inst)
        if changed:
            bb.instructions = out
    return n_split


def build_nc(cfg: Cfg, for_sim=False):
    from contextlib import ExitStack

    nc = bass.Bass("TRN2", target_bir_lowering=False, debug=False)
    f8, f16, f32 = dt.float8e4, dt.float16, dt.float32
    io = {
        "ct": nc.dram_tensor("ct", [128, cfg.CKN, cfg.rows], f16, kind="ExternalInput").ap(),
        "qt2w": nc.dram_tensor("qt2w", [128, cfg.CKN, cfg.d], f16, kind="ExternalInput").ap(),
        "kb4": nc.dram_tensor("kb4", [cfg.MC, 128, cfg.CKN, cfg.MB], f8, kind="ExternalInput").ap(),
        "nkb2": nc.dram_tensor("nkb2", [cfg.MC, 1, 2, cfg.MB], f8, kind="ExternalInput").ap(),
        "kbrows8": nc.dram_tensor("kbrows8", [cfg.nkb, cfg.d], f8, kind="ExternalInput").ap(),
        "cbase": nc.dram_tensor("cbase", [128, cfg.CAND], f32, kind="ExternalInput").ap(),
        "biasw": nc.dram_tensor("biasw", [1, 2, 128], f8, kind="ExternalInput").ap(),
        "eye": nc.dram_tensor("eye", [128, 128], f16, kind="ExternalInput").ap(),
        "crows": nc.dram_tensor("crows", [cfg.rows, cfg.d], f32, kind="ExternalInput").ap(),
        "krows": nc.dram_tensor("krows", [cfg.rows, cfg.d], f32, kind="ExternalInput").ap(),
        "out": nc.dram_tensor("out", [cfg.rows, 3 * cfg.d], f32, kind="ExternalOutput").ap(),
    }
    if OPTS.get("dbg"):
        u16_, u32_ = dt.uint16, dt.uint32
        io["dbg_cand"] = nc.dram_tensor("dbg_cand", [128, cfg.CAND], f16, kind="ExternalOutput").ap()
        io["dbg_cpos"] = nc.dram_tensor("dbg_cpos", [128, cfg.CAND], u16_, kind="ExternalOutput").ap()
        io["dbg_wv"] = nc.dram_tensor("dbg_wv", [128, cfg.topk], f32, kind="ExternalOutput").ap()
        io["dbg_gidx"] = nc.dram_tensor("dbg_gidx", [128, cfg.topk], u32_, kind="ExternalOutput").ap()
        io["dbg_dist"] = nc.dram_tensor("dbg_dist", [128, cfg.topk], f32, kind="ExternalOutput").ap()
        io["dbg_v"] = nc.dram_tensor("dbg_v", [128, 2, 4, 256], f16, kind="ExternalOutput").ap()
    reps = int(OPTS.get("reps", 1))
    with TileContext(nc) as tc:
        for _ in range(reps):
            with ExitStack() as ctx:
                build_body(tc, io, cfg, ctx)
    if not for_sim:
        split_sync_waits(nc, limit=1)
    return nc


def host_prep(C, K, tie_kb, Q_weight, cfg: Cfg, n_cores=N_CORES):
    """Build per-core input maps from full inputs."""
    import ml_dtypes

    F8 = ml_dtypes.float8_e4m3
    C = np.asarray(C, dtype=np.float32)
    K = np.asarray(K, dtype=np.float32)
    tie_kb = np.asarray(tie_kb, dtype=np.float32)
    Q_weight = np.asarray(Q_weight, dtype=np.float32)
    d, rows, MC, MB, CKN = cfg.d, cfg.rows, cfg.MC, cfg.MB, cfg.CKN

    kb8 = tie_kb.astype(F8)
    # kb4[c, p, ck, m] = kb8[c*MB+m, ck*128+p]
    kb4 = np.ascontiguousarray(
        kb8.reshape(MC, MB, CKN, 128).transpose(0, 3, 2, 1)
    )
    kbsq = (tie_kb.astype(np.float32) ** 2).sum(axis=1)
    vbias = np.float32(d) - kbsq.astype(np.float32)
    hi = np.round(vbias / 32.0).astype(F8)
    lo = (vbias - 32.0 * hi.astype(np.float32)).astype(F8)
    nkb2 = np.stack([hi, lo], axis=0).reshape(2, MC, MB).transpose(1, 0, 2)
    nkb2 = np.ascontiguousarray(nkb2.reshape(MC, 1, 2, MB))
    biasw = np.empty((1, 2, 128), dtype=F8)
    biasw[0, 0, :] = F8(32.0)
    biasw[0, 1, :] = F8(1.0)
    # qt2w[p, ck, c'] = 2*Q_weight[c', ck*128+p]
    qt2w = np.ascontiguousarray(
        (2.0 * Q_weight.T).astype(np.float16).reshape(CKN, 128, d).transpose(1, 0, 2)
    )
    cbase = np.broadcast_to(
        ((np.arange(cfg.CAND, dtype=np.float32) // 8) * 512.0), (128, cfg.CAND)
    ).copy()
    eye = np.eye(128, dtype=np.float16)

    in_maps = []
    for i in range(n_cores):
        Cs = C[i * rows : (i + 1) * rows]
        Ks = K[i * rows : (i + 1) * rows]
        ct = np.ascontiguousarray(
            Cs.T.astype(np.float16).reshape(CKN, 128, rows).transpose(1, 0, 2)
        )
        in_maps.append(
            {
                "ct": ct,
                "qt2w": qt2w,
                "kb4": kb4,
                "nkb2": nkb2,
                "kbrows8": kb8,
                "cbase": cbase,
                "biasw": biasw,
                "eye": eye,
                "crows": np.ascontiguousarray(Cs),
                "krows": np.ascontiguousarray(Ks),
            }
        )
    return in_maps


_NC_CACHE = {}


def kernel(C, K, tie_kb, Q_weight, temperature=1.0, top_k=32):
    from concourse.bass_utils import run_bass_kernel_spmd

    C = np.asarray(C)
    n = C.shape[0]
    cfg = Cfg(
        rows=n // N_CORES,
        d=C.shape[1],
        nkb=np.asarray(tie_kb).shape[0],
        topk=int(top_k),
        temperature=float(temperature),
    )
    key = (cfg.rows, cfg.d, cfg.nkb, cfg.topk, cfg.temp)
    if key not in _NC_CACHE:
        _NC_CACHE[key] = build_nc(cfg)
    nc = _NC_CACHE[key]
    in_maps = host_prep(C, K, tie_kb, Q_weight, cfg)
    res = run_bass_kernel_spmd(nc, in_maps, core_ids=list(range(N_CORES)))
    return np.concatenate([res.results[i]["out"] for i in range(N_CORES)], axis=0)
